# revision 1
# baseline (speedup 1.0000x reference)
"""BiasAdd + LayerNorm + FP8(E4M3) quantize, fused, sharded over 8 NeuronCores.

Contract: kernel(**inputs) takes FULL unsharded inputs (as in setup_inputs()),
returns (bda_out[N,D] f32, ln_out_fp8[N,D] f32 (dequantized e4m3fn values),
amax scalar f32) exactly like the reference.

Sharding: rows (tokens) split evenly across 8 cores; per-hidden vectors
(bias, ln_weight, ln_bias) replicated; per-core partial amax maxed on host
(the unshard step of a sharded max).
"""
import sys
import os

for _p in ("/opt/trn_rl_repo",):
    if _p not in sys.path and os.path.isdir(_p):
        sys.path.insert(0, _p)

from contextlib import ExitStack

import numpy as np

# ---- hardcoded problem geometry (from the problem spec) ----
N_TOKENS = 65536
D = 1024
N_CORES = 8
TPC = N_TOKENS // N_CORES          # tokens per core = 8192
P = 128                            # SBUF partitions
T_SUB = 8                          # tokens per partition per chunk
CHUNK_T = P * T_SUB                # tokens per chunk = 1024
N_CHUNKS = TPC // CHUNK_T          # 8
EPS = 1e-5

_CACHE = {}
LAST_RESULTS = None  # BassKernelResults of the most recent run (for test harnesses)


def _rep_ap(tile_ap, t_sub):
    """View a [P, D] SBUF tile as [P, t_sub, D] with a 0-stride middle dim."""
    import concourse.bass as bass
    a = tile_ap
    assert len(a.ap) == 2, a.ap
    return bass.AP(tensor=a.tensor, offset=a.offset,
                   ap=[list(a.ap[0]), [0, t_sub], list(a.ap[1])])


def _bcast_dram_ap(dram_ap, p):
    """View a [D] DRAM vector as [p, D] via a 0-stride partition dim."""
    import concourse.bass as bass
    a = dram_ap
    return bass.AP(tensor=a.tensor, offset=a.offset, ap=[[0, p], list(a.ap[0])])


def build_nc(tpc=TPC, t_sub=T_SUB):
    """Build + compile the per-core Bass program (SPMD: same program, 8 cores)."""
    import concourse.bacc as bacc
    import concourse.tile as tile
    from concourse import mybir, bass_isa

    n_chunks = tpc // (P * t_sub)
    assert n_chunks * P * t_sub == tpc

    nc = bacc.Bacc("TRN2", target_bir_lowering=False, debug=False)

    x_d = nc.dram_tensor("x", [tpc, D], mybir.dt.float32, kind="ExternalInput")
    res_d = nc.dram_tensor("residual", [tpc, D], mybir.dt.float32, kind="ExternalInput")
    bias_d = nc.dram_tensor("bias", [D], mybir.dt.float32, kind="ExternalInput")
    gamma_d = nc.dram_tensor("ln_weight", [D], mybir.dt.float32, kind="ExternalInput")
    beta_d = nc.dram_tensor("ln_bias", [D], mybir.dt.float32, kind="ExternalInput")
    bda_d = nc.dram_tensor("bda_out", [tpc, D], mybir.dt.float32, kind="ExternalOutput")
    fp8_d = nc.dram_tensor("ln_fp8", [tpc, D], mybir.dt.float8e4, kind="ExternalOutput")
    amax_d = nc.dram_tensor("amax", [1, 1], mybir.dt.float32, kind="ExternalOutput")

    AF = mybir.ActivationFunctionType
    OP = mybir.AluOpType

    with tile.TileContext(nc) as tc, ExitStack() as ctx:
        consts = ctx.enter_context(tc.tile_pool(name="consts", bufs=1))
        io = ctx.enter_context(tc.tile_pool(name="io", bufs=2))
        stats = ctx.enter_context(tc.tile_pool(name="stats", bufs=2))
        accp = ctx.enter_context(tc.tile_pool(name="accp", bufs=1))

        bias_b = consts.tile([P, D], mybir.dt.float32)
        gamma_b = consts.tile([P, D], mybir.dt.float32)
        beta_b = consts.tile([P, D], mybir.dt.float32)
        nc.sync.dma_start(out=bias_b[:], in_=_bcast_dram_ap(bias_d.ap(), P))
        nc.sync.dma_start(out=gamma_b[:], in_=_bcast_dram_ap(gamma_d.ap(), P))
        nc.sync.dma_start(out=beta_b[:], in_=_bcast_dram_ap(beta_d.ap(), P))

        eps_t = consts.tile([P, 1], mybir.dt.float32)
        nc.vector.memset(eps_t[:], EPS)
        amax_slots = accp.tile([P, n_chunks], mybir.dt.float32)
        amax_acc = accp.tile([P, 1], mybir.dt.float32)

        x_v = x_d.ap().rearrange("(c p t) d -> c p (t d)", p=P, t=t_sub)
        res_v = res_d.ap().rearrange("(c p t) d -> c p (t d)", p=P, t=t_sub)
        bda_v = bda_d.ap().rearrange("(c p t) d -> c p (t d)", p=P, t=t_sub)
        fp8_v = fp8_d.ap().rearrange("(c p t) d -> c p (t d)", p=P, t=t_sub)

        n_half = D // 512

        for c in range(n_chunks):
            x_t = io.tile([P, t_sub, D], mybir.dt.float32, tag="x_t")
            r_t = io.tile([P, t_sub, D], mybir.dt.float32, tag="r_t")
            q_t = io.tile([P, t_sub, D], mybir.dt.float8e4, tag="q_t")
            nc.sync.dma_start(out=x_t[:].rearrange("p t d -> p (t d)"), in_=x_v[c])
            nc.sync.dma_start(out=r_t[:].rearrange("p t d -> p (t d)"), in_=res_v[c])

            # bda = (x + bias) + residual  (matches reference add order exactly)
            nc.vector.tensor_tensor(out=x_t[:], in0=x_t[:], in1=_rep_ap(bias_b[:], t_sub), op=OP.add)
            nc.gpsimd.tensor_tensor(out=x_t[:], in0=x_t[:], in1=r_t[:], op=OP.add)
            nc.sync.dma_start(out=bda_v[c], in_=x_t[:].rearrange("p t d -> p (t d)"))

            # per-token mean/var via bn_stats over 512-wide subgroups
            st = stats.tile([P, t_sub, n_half, 6], mybir.dt.float32, tag="st")
            x_view = x_t[:].rearrange("p t (h f) -> p t h f", f=512)
            for t_i in range(t_sub):
                for h in range(n_half):
                    nc.vector.bn_stats(out=st[:, t_i, h, :], in_=x_view[:, t_i, h, :])
            mv = stats.tile([P, t_sub, 2], mybir.dt.float32, tag="mv")
            for t_i in range(t_sub):
                nc.vector.bn_aggr(out=mv[:, t_i, :], in_=st[:, t_i, :, :])

            # rsig = 1/sqrt(var + eps); nmu = -mu
            s_t = stats.tile([P, t_sub], mybir.dt.float32, tag="s_t")
            r_sig = stats.tile([P, t_sub], mybir.dt.float32, tag="r_sig")
            nmu = stats.tile([P, t_sub], mybir.dt.float32, tag="nmu")
            nc.scalar.activation(out=s_t[:], in_=mv[:, :, 1], func=AF.Sqrt,
                                 bias=eps_t[:, 0:1], scale=1.0)
            nc.vector.reciprocal(out=r_sig[:], in_=s_t[:])
            nc.vector.tensor_scalar(out=nmu[:], in0=mv[:, :, 0], scalar1=-1.0,
                                    scalar2=None, op0=OP.mult)

            # u = bda - mu ; y0 = u * rsig   (two exact passes on ScalarE)
            for t_i in range(t_sub):
                nc.scalar.activation(out=r_t[:, t_i, :], in_=x_t[:, t_i, :],
                                     func=AF.Identity, bias=nmu[:, t_i:t_i + 1], scale=1.0)
                nc.scalar.activation(out=r_t[:, t_i, :], in_=r_t[:, t_i, :],
                                     func=AF.Identity, bias=0.0, scale=r_sig[:, t_i:t_i + 1])

            # y = y0 * gamma + beta
            nc.gpsimd.tensor_tensor(out=r_t[:], in0=r_t[:], in1=_rep_ap(gamma_b[:], t_sub), op=OP.mult)
            nc.vector.tensor_tensor(out=r_t[:], in0=r_t[:], in1=_rep_ap(beta_b[:], t_sub), op=OP.add)

            # per-chunk absmax, fp8 quantize, store
            nc.vector.tensor_reduce(out=amax_slots[:, c:c + 1], in_=r_t[:],
                                    axis=mybir.AxisListType.XY, op=OP.max,
                                    apply_absolute_value=True)
            nc.scalar.activation(out=q_t[:], in_=r_t[:], func=AF.Copy)
            nc.sync.dma_start(out=fp8_v[c], in_=q_t[:].rearrange("p t d -> p (t d)"))

        # fold chunk slots, then cross-partition absmax, store scalar
        nc.vector.tensor_reduce(out=amax_acc[:], in_=amax_slots[:],
                                axis=mybir.AxisListType.X, op=OP.max,
                                apply_absolute_value=True)
        amax_all = accp.tile([P, 1], mybir.dt.float32)
        nc.gpsimd.partition_all_reduce(amax_all[:], amax_acc[:], channels=P,
                                       reduce_op=bass_isa.ReduceOp.absmax)
        nc.sync.dma_start(out=amax_d.ap(), in_=amax_all[0:1, 0:1])

    nc.compile()
    return nc


def kernel(x, bias, residual, ln_weight, ln_bias):
    global LAST_RESULTS
    from concourse.bass_utils import run_bass_kernel_spmd

    nc = _CACHE.get("nc")
    if nc is None:
        nc = _CACHE["nc"] = build_nc()

    x = np.ascontiguousarray(np.asarray(x, dtype=np.float32))
    residual = np.ascontiguousarray(np.asarray(residual, dtype=np.float32))
    bias = np.ascontiguousarray(np.asarray(bias, dtype=np.float32))
    ln_weight = np.ascontiguousarray(np.asarray(ln_weight, dtype=np.float32))
    ln_bias = np.ascontiguousarray(np.asarray(ln_bias, dtype=np.float32))
    assert x.shape == (N_TOKENS, D) and residual.shape == (N_TOKENS, D)

    xs = x.reshape(N_CORES, TPC, D)
    rs = residual.reshape(N_CORES, TPC, D)
    in_maps = [
        {"x": xs[k], "residual": rs[k], "bias": bias,
         "ln_weight": ln_weight, "ln_bias": ln_bias}
        for k in range(N_CORES)
    ]

    res = run_bass_kernel_spmd(nc, in_maps, list(range(N_CORES)))
    LAST_RESULTS = res
    outs = res.results

    bda = np.concatenate([np.asarray(outs[k]["bda_out"]) for k in range(N_CORES)], axis=0)
    fp8 = np.concatenate(
        [np.asarray(outs[k]["ln_fp8"]).astype(np.float32) for k in range(N_CORES)], axis=0)
    amax = np.float32(max(float(np.asarray(outs[k]["amax"]).reshape(-1)[0])
                          for k in range(N_CORES)))
    return bda, fp8, amax
